# revision 1
# baseline (speedup 1.0000x reference)
"""GatedGraphConvolution Trainium2 kernel.

out = input + segment_sum(sigmoid(g) * e, edge_sources)
  where [g|e] = input[edge_targets] @ W.T

Key algebraic fact: the per-edge message depends ONLY on the target node:
  msg_e = M[target_e],  M[n] = sigmoid(x_n @ Wg.T) * (x_n @ We.T)
so we (phase A) compute the dense M table [N, F] once per core, and
(phase B) gather M rows per edge + scatter-add by source via one-hot
matmuls accumulated in PSUM.

Sharding: nodes are sharded by SOURCE across the 8 cores (6250 nodes each);
each core receives exactly the edges whose source is in its slice, so there
are no collectives.  Edges are sorted by 128-node source "window"; each
window's edges accumulate into one PSUM tile via lhsT=onehot matmuls.
The dma_gather int16 index limit (<=32767) is handled by splitting each
window's edges into low-target (< 32768) and high-target tiles and issuing
gathers against two base offsets of the M table.
"""

import math
import os
import sys
from dataclasses import dataclass, field

import numpy as np

if "/opt/trn_rl_repo" not in sys.path:
    sys.path.insert(0, "/opt/trn_rl_repo")

import ml_dtypes

P = 128  # partitions / tile edge
F = 128  # feature dim (OUT_F == IN_F == 128)
TF = 2 * F

BF16 = ml_dtypes.bfloat16


@dataclass
class Cfg:
    n_nodes: int = 50000
    n_cores: int = 8
    half: int = 32768  # int16 index limit boundary (multiple of 128)
    gw: int = 5  # windows per gather group
    ca: int = 16  # node-tiles per phase-A chunk

    @property
    def na(self) -> int:  # node tiles in M table
        return math.ceil(self.n_nodes / P)

    @property
    def npc(self) -> int:  # nodes per core
        assert self.n_nodes % self.n_cores == 0
        return self.n_nodes // self.n_cores

    @property
    def nwin(self) -> int:  # source windows per core
        return math.ceil(self.npc / P)


@dataclass
class Plan:
    """Static schedule shared by all cores + per-core host arrays."""

    T_lo: list  # tiles per (window, low-half), max over cores
    T_hi: list
    groups: list = field(default_factory=list)  # (ws, lo_tiles, hi_tiles)
    tiles_of: dict = field(default_factory=dict)  # (w, half) -> [tile ids]
    win_tiles: dict = field(default_factory=dict)  # w -> [(tile, half, pos_in_buf)]
    TT: int = 0
    # per-core packed arrays
    srel: list = field(default_factory=list)  # [P, TT] bf16
    gidx: list = field(default_factory=list)  # [P, 8*TT] int16


def _make_schedule(cfg: Cfg, T_lo, T_hi) -> Plan:
    plan = Plan(T_lo=T_lo, T_hi=T_hi)
    t = 0
    for g0 in range(0, cfg.nwin, cfg.gw):
        ws = list(range(g0, min(g0 + cfg.gw, cfg.nwin)))
        lo, hi = [], []
        for w in ws:
            for _ in range(T_lo[w]):
                plan.tiles_of.setdefault((w, 0), []).append(t)
                lo.append((w, t))
                t += 1
        for w in ws:
            for _ in range(T_hi[w]):
                plan.tiles_of.setdefault((w, 1), []).append(t)
                hi.append((w, t))
                t += 1
        plan.groups.append((ws, lo, hi))
    plan.TT = t
    # per window: list of (global tile id, half, position within the group's
    # lo/hi gather buffer) in mm2 consumption order
    for ws, lo, hi in plan.groups:
        for w in ws:
            lst = []
            for pos, (w2, t2) in enumerate(lo):
                if w2 == w:
                    lst.append((t2, 0, pos))
            for pos, (w2, t2) in enumerate(hi):
                if w2 == w:
                    lst.append((t2, 1, pos))
            plan.win_tiles[w] = lst
    return plan


def _plan(cfg: Cfg, edge_sources: np.ndarray, edge_targets: np.ndarray) -> Plan:
    src = edge_sources.astype(np.int64)
    tgt = edge_targets.astype(np.int64)
    npc, nwin = cfg.npc, cfg.nwin

    # bucket edges per (core, window, half)
    core = src // npc
    w_all = (src % npc) // P
    srel_all = (src % npc) % P
    hi_all = (tgt >= cfg.half).astype(np.int64)

    counts = np.zeros((cfg.n_cores, nwin, 2), np.int64)
    np.add.at(counts, (core, w_all, hi_all), 1)
    tmax = counts.max(axis=0)  # [nwin, 2]
    T_lo = [int(math.ceil(tmax[w, 0] / P)) for w in range(nwin)]
    T_hi = [int(math.ceil(tmax[w, 1] / P)) for w in range(nwin)]

    plan = _make_schedule(cfg, T_lo, T_hi)

    # pack per-core slot arrays
    order = np.lexsort((hi_all, w_all, core))
    src_s, w_s, srel_s, hi_s, tgt_s = (
        src[order],
        w_all[order],
        srel_all[order],
        hi_all[order],
        tgt[order],
    )
    bounds = {}
    keys = core[order] * (nwin * 2) + w_s * 2 + hi_s
    uniq, starts = np.unique(keys, return_index=True)
    starts = list(starts) + [len(keys)]
    for i, k in enumerate(uniq):
        bounds[int(k)] = (starts[i], starts[i + 1])

    for c in range(cfg.n_cores):
        srel_arr = np.full((plan.TT * P,), 255.0, np.float32)
        gidx_arr = np.zeros((plan.TT * P,), np.int16)
        for w in range(nwin):
            for h in (0, 1):
                k = c * (nwin * 2) + w * 2 + h
                if k not in bounds:
                    continue
                a, b = bounds[k]
                tiles = plan.tiles_of.get((w, h), [])
                assert (b - a) <= len(tiles) * P
                sr = srel_s[a:b]
                tg = tgt_s[a:b] - (cfg.half if h else 0)
                for i in range(b - a):
                    t = tiles[i // P]
                    j = i % P
                    s = t * P + j
                    srel_arr[s] = sr[i]
                    gidx_arr[s] = tg[i]
        srel_host = srel_arr.reshape(plan.TT, P).T.astype(BF16)  # [P, TT]
        g16 = gidx_arr.reshape(plan.TT * 8, 16).T  # [16, TT*8]
        gidx_host = np.tile(g16, (8, 1)).astype(np.int16)  # [P, TT*8]
        plan.srel.append(np.ascontiguousarray(srel_host))
        plan.gidx.append(np.ascontiguousarray(gidx_host))
    return plan


def _host_arrays(cfg: Cfg, inp: np.ndarray, W: np.ndarray):
    """Replicated input arrays: xT (transposed node features), wT, iota."""
    n = cfg.n_nodes
    xT = np.zeros((P, cfg.na * P), np.float32)
    xT[:, :n] = inp.T
    wT = np.ascontiguousarray(W.T)  # [F, 2F]
    iota = np.tile(np.arange(P, dtype=np.float32), (P, 1))
    return (
        np.ascontiguousarray(xT.astype(BF16)),
        np.ascontiguousarray(wT.astype(BF16)),
        np.ascontiguousarray(iota.astype(BF16)),
    )


def _xs_core(cfg: Cfg, inp: np.ndarray, c: int) -> np.ndarray:
    """Per-core input slice packed [P, nwin*F]: xs[p, w*F+f] = x[c*npc+w*P+p, f]."""
    npc, nwin = cfg.npc, cfg.nwin
    sl = np.zeros((nwin * P, F), np.float32)
    sl[:npc] = inp[c * npc : (c + 1) * npc]
    return np.ascontiguousarray(sl.reshape(nwin, P, F).transpose(1, 0, 2).reshape(P, nwin * F))


def _build(cfg: Cfg, plan: Plan, enable_asserts: bool = False):
    import concourse.bacc as bacc
    import concourse.tile as tile
    from concourse import mybir

    nc = bacc.Bacc(
        "TRN2",
        target_bir_lowering=False,
        debug=False,
        enable_asserts=enable_asserts,
        num_devices=cfg.n_cores,
    )
    dt = mybir.dt

    na, nwin, npc = cfg.na, cfg.nwin, cfg.npc
    TT = plan.TT

    xT_d = nc.dram_tensor("xT", [P, na * P], dt.bfloat16, kind="ExternalInput")
    wT_d = nc.dram_tensor("wT", [P, TF], dt.bfloat16, kind="ExternalInput")
    iota_d = nc.dram_tensor("iota", [P, P], dt.bfloat16, kind="ExternalInput")
    xs_d = nc.dram_tensor("xs", [P, nwin * F], dt.float32, kind="ExternalInput")
    srel_d = nc.dram_tensor("srel", [P, TT], dt.bfloat16, kind="ExternalInput")
    gidx_d = nc.dram_tensor("gidx", [P, 8 * TT], dt.int16, kind="ExternalInput")
    y_d = nc.dram_tensor("y", [npc, F], dt.float32, kind="ExternalOutput")
    mtab_d = nc.dram_tensor("mtab", [na * P, F], dt.bfloat16, kind="Internal")

    lo_rows = min(cfg.half, na * P)

    max_tl = max((len(lo) for _, lo, _ in plan.groups), default=0)
    max_th = max((len(hi) for _, _, hi in plan.groups), default=0)
    max_tg = max((len(lo) + len(hi) for _, lo, hi in plan.groups), default=0)

    n_chunks = math.ceil(na / cfg.ca)

    from concourse.tile import add_dep_helper

    with tile.TileContext(nc) as tc:
        import contextlib

        with contextlib.ExitStack() as ctx:
            consts = ctx.enter_context(tc.tile_pool(name="consts", bufs=1))
            a_in = ctx.enter_context(tc.tile_pool(name="a_in", bufs=3))
            a_ps = ctx.enter_context(tc.tile_pool(name="a_ps", bufs=4, space="PSUM"))
            a_sg = ctx.enter_context(tc.tile_pool(name="a_sg", bufs=4))
            a_m = ctx.enter_context(tc.tile_pool(name="a_m", bufs=3))
            b_lo = ctx.enter_context(tc.tile_pool(name="b_lo", bufs=2))
            b_hi = ctx.enter_context(tc.tile_pool(name="b_hi", bufs=2))
            b_oh = ctx.enter_context(tc.tile_pool(name="b_oh", bufs=2))
            b_ps = ctx.enter_context(tc.tile_pool(name="b_ps", bufs=2, space="PSUM"))
            b_out = ctx.enter_context(tc.tile_pool(name="b_out", bufs=2))

            # ---- constants to SBUF ----
            wT_sb = consts.tile([P, TF], dt.bfloat16, tag="wT")
            nc.sync.dma_start(wT_sb[:], wT_d[:, :])
            iota_sb = consts.tile([P, P], dt.bfloat16, tag="iota")
            nc.sync.dma_start(iota_sb[:], iota_d[:, :])
            xs_sb = consts.tile([P, nwin * F], dt.float32, tag="xs")
            nc.sync.dma_start(xs_sb[:], xs_d[:, :])
            srel_sb = consts.tile([P, TT], dt.bfloat16, tag="srel")
            nc.sync.dma_start(srel_sb[:], srel_d[:, :])
            gidx_sb = consts.tile([P, 8 * TT], dt.int16, tag="gidx")
            nc.sync.dma_start(gidx_sb[:], gidx_d[:, :])

            # ---- phase A: M table ----
            mdmas = []
            for ci in range(n_chunks):
                c0 = ci * cfg.ca
                ca = min(cfg.ca, na - c0)
                xt = a_in.tile([P, cfg.ca * P], dt.bfloat16, tag="xt")
                nc.sync.dma_start(xt[:, : ca * P], xT_d[:, c0 * P : (c0 + ca) * P])
                mtile = a_m.tile([P, cfg.ca * F], dt.bfloat16, tag="mtile")
                for k in range(ca):
                    ps = a_ps.tile([P, TF], dt.float32, tag="psA")
                    nc.tensor.matmul(
                        ps[:],
                        lhsT=xt[:, k * P : (k + 1) * P],
                        rhs=wT_sb[:],
                        start=True,
                        stop=True,
                    )
                    sg = a_sg.tile([P, F], dt.float32, tag="sg")
                    nc.scalar.activation(
                        sg[:], ps[:, 0:F], mybir.ActivationFunctionType.Sigmoid
                    )
                    nc.vector.tensor_mul(
                        mtile[:, k * F : (k + 1) * F], ps[:, F:TF], sg[:]
                    )
                out_ap = (
                    mtab_d[c0 * P : (c0 + ca) * P, :]
                    .rearrange("(k p) f -> p k f", p=P)
                )
                mdmas.append(
                    nc.sync.dma_start(
                        out_ap, mtile[:, : ca * F].rearrange("p (k f) -> p k f", f=F)
                    )
                )

            # ---- phase B: gather + one-hot scatter ----
            for ws, lo, hi in plan.groups:
                lob = hib = None
                if lo:
                    t0 = lo[0][1]
                    tl = len(lo)
                    lob = b_lo.tile([P, max(max_tl, 1) * F], dt.bfloat16, tag="lob")
                    g = nc.gpsimd.dma_gather(
                        out_ap=lob[:, : tl * F].rearrange("p (t e) -> p t e", e=F),
                        in_ap=mtab_d[0:lo_rows, :],
                        idxs_ap=gidx_sb[:, 8 * t0 : 8 * (t0 + tl)],
                        num_idxs=tl * P,
                        num_idxs_reg=tl * P,
                        elem_size=F,
                        single_packet=False,
                    )
                    for m in mdmas:
                        add_dep_helper(g.ins, m.ins, reason="mtab RAW")
                if hi:
                    t0 = hi[0][1]
                    th = len(hi)
                    hib = b_hi.tile([P, max(max_th, 1) * F], dt.bfloat16, tag="hib")
                    g = nc.gpsimd.dma_gather(
                        out_ap=hib[:, : th * F].rearrange("p (t e) -> p t e", e=F),
                        in_ap=mtab_d[cfg.half : na * P, :],
                        idxs_ap=gidx_sb[:, 8 * t0 : 8 * (t0 + th)],
                        num_idxs=th * P,
                        num_idxs_reg=th * P,
                        elem_size=F,
                        single_packet=False,
                    )
                    for m in mdmas:
                        add_dep_helper(g.ins, m.ins, reason="mtab RAW")

                # one-hot for the whole group in one DVE op
                tg0 = (lo + hi)[0][1] if (lo or hi) else None
                ntg = len(lo) + len(hi)
                oh = None
                if ntg:
                    oh = b_oh.tile([P, max(max_tg, 1) * P], dt.bfloat16, tag="oh")
                    nc.vector.tensor_tensor(
                        out=oh[:, : ntg * P].rearrange("p (t e) -> p t e", e=P),
                        in0=srel_sb[:, tg0 : tg0 + ntg]
                        .unsqueeze(2)
                        .to_broadcast([P, ntg, P]),
                        in1=iota_sb[:].unsqueeze(1).to_broadcast([P, ntg, P]),
                        op=mybir.AluOpType.is_equal,
                    )

                for w in ws:
                    tiles = plan.win_tiles.get(w, [])
                    rows = min(P, npc - w * P)
                    ot = b_out.tile([P, F], dt.float32, tag="ot")
                    if not tiles:
                        nc.vector.tensor_copy(ot[:], xs_sb[:, w * F : (w + 1) * F])
                    else:
                        ps = b_ps.tile([P, F], dt.float32, tag="psB")
                        for i, (t, h, pos) in enumerate(tiles):
                            buf = hib if h else lob
                            nc.tensor.matmul(
                                ps[:],
                                lhsT=oh[:, (t - tg0) * P : (t - tg0 + 1) * P],
                                rhs=buf[:, pos * F : (pos + 1) * F],
                                start=(i == 0),
                                stop=(i == len(tiles) - 1),
                            )
                        nc.vector.tensor_add(
                            ot[:], ps[:], xs_sb[:, w * F : (w + 1) * F]
                        )
                    nc.sync.dma_start(y_d[w * P : w * P + rows, :], ot[:rows, :])

    nc.compile()
    return nc


def _in_maps(cfg: Cfg, plan: Plan, inp: np.ndarray, W: np.ndarray):
    xT, wT, iota = _host_arrays(cfg, inp, W)
    maps = []
    for c in range(cfg.n_cores):
        maps.append(
            {
                "xT": xT,
                "wT": wT,
                "iota": iota,
                "xs": _xs_core(cfg, inp, c),
                "srel": plan.srel[c],
                "gidx": plan.gidx[c],
            }
        )
    return maps


def _install_ntff_hook():
    """Provide the antenv.axon_hooks shim trn_boot expects, so trace=True
    can capture NTFF profiles. Silently degrades if anything is missing."""
    try:
        import antenv.axon_hooks  # noqa: F401

        return
    except ImportError:
        pass
    try:
        import types

        import antenv

        mod = types.ModuleType("antenv.axon_hooks")
        _hook = [None]
        mod.set_axon_ntff_profile_hook = lambda h: _hook.__setitem__(0, h)
        mod.get_axon_ntff_profile_hook = lambda: _hook[0]
        sys.modules["antenv.axon_hooks"] = mod
        antenv.axon_hooks = mod
        from trn_agent_boot import trn_boot

        mod.set_axon_ntff_profile_hook(
            trn_boot._ntff_profile_via_ctypes("/opt/axon/libaxon_pjrt.so")
        )
    except Exception:
        pass


def kernel(**inputs) -> np.ndarray:
    inp = np.asarray(inputs["input"], np.float32)
    W = np.asarray(inputs["W"], np.float32)
    es = np.asarray(inputs["edge_sources"]).astype(np.int64)
    et = np.asarray(inputs["edge_targets"]).astype(np.int64)

    cfg = Cfg(n_nodes=inp.shape[0])
    plan = _plan(cfg, es, et)
    nc = _build(cfg, plan)

    from concourse.bass_utils import run_bass_kernel_spmd

    if bool(int(os.environ.get("GGC_TRACE", "0"))):
        _install_ntff_hook()
    res = run_bass_kernel_spmd(
        nc,
        _in_maps(cfg, plan, inp, W),
        core_ids=list(range(cfg.n_cores)),
        trace=bool(int(os.environ.get("GGC_TRACE", "0"))),
    )
    out = np.concatenate([res.results[c]["y"] for c in range(cfg.n_cores)], axis=0)
    if bool(int(os.environ.get("GGC_TRACE", "0"))):
        kernel.last_results = res  # stash for test harness
    return out



# revision 4
# speedup vs baseline: 2.9291x; 2.9291x over previous
"""GatedGraphConvolution Trainium2 kernel (host-gather edition).

out = input + segment_sum(sigmoid(g) * e, edge_sources)
  where [g|e] = input[edge_targets] @ W.T

Sharding: edges are sharded by SOURCE node across the 8 cores (6250 nodes
each) so per-core outputs are disjoint and no collectives are needed.  The
host pre-gathers input[edge_targets] for each core's edges ("gathered rows"
per the sharding hint), sorted by 128-node source window and padded to
128-edge tiles.

Device per tile (128 edges):
  mm1:   ps[e, 0:256] = xgT_tile.T @ W.T            (one matmul)
  act:   sg = sigmoid(ps[:, 0:128])                  (batched 2 tiles/instr)
  mul:   msg = sg * ps[:, 128:256]                   (DVE, bf16 out)
  mm2:   psB[srel, f] += onehot(srel)_tile.T @ msg   (PSUM accum per window)
window end: y = psB + x_slice -> DRAM.

This removes the dma_gather entirely (SWDGE descriptor generation measured
~8ns/row on GPSIMD = 854us serial in the M-table design) at the cost of
per-edge rather than per-node W-matmuls.
"""

import math
import os
import sys
from collections import deque

import numpy as np

if "/opt/trn_rl_repo" not in sys.path:
    sys.path.insert(0, "/opt/trn_rl_repo")

import ml_dtypes

P = 128  # partitions / tile edge
F = 128  # feature dim (OUT_F == IN_F == 128)
TF = 2 * F

BF16 = ml_dtypes.bfloat16

N_NODES = 50000
N_CORES = 8
NPC = N_NODES // N_CORES  # 6250
NWIN = math.ceil(NPC / P)  # 49
CH = 16  # tiles per xgT DMA chunk
PAIR = 2  # tiles per PSUM bank (psGE [128, 512] holds 2x256)
MM2_LAG = 4  # tiles of lag before emitting scatter matmuls


def _plan(edge_sources: np.ndarray):
    """Per-core, per-window tile schedule. Returns (T_w, toff, TT, win_of)."""
    src = edge_sources.astype(np.int64)
    core = src // NPC
    w_all = (src % NPC) // P
    counts = np.zeros((N_CORES, NWIN), np.int64)
    np.add.at(counts, (core, w_all), 1)
    tmax = counts.max(axis=0)
    T_w = [int(math.ceil(tmax[w] / P)) for w in range(NWIN)]
    toff = np.concatenate([[0], np.cumsum(T_w)]).astype(np.int64)
    TT = int(toff[-1])
    win_of = np.zeros(TT, np.int64)
    for w in range(NWIN):
        win_of[toff[w] : toff[w + 1]] = w
    return T_w, toff, TT, win_of


def _host_core_arrays(inp16, inp, edge_sources, edge_targets, toff, TT, c):
    """xgT [P, TT*P] bf16, srel [P, TT] bf16, xs [P, NWIN*F] f32 for core c."""
    src = edge_sources.astype(np.int64)
    tgt = edge_targets.astype(np.int64)
    sel = (src // NPC) == c
    src_c = src[sel] - c * NPC
    tgt_c = tgt[sel]
    w_c = src_c // P
    srel_c = src_c % P

    order = np.argsort(w_c, kind="stable")
    w_s = w_c[order]
    srel_s = srel_c[order]
    tgt_s = tgt_c[order]

    counts = np.bincount(w_s, minlength=NWIN)
    cum = np.concatenate([[0], np.cumsum(counts)])
    within = np.arange(len(w_s)) - cum[w_s]
    pos = toff[w_s] * P + within  # slot index in [0, TT*P)

    srel_arr = np.full(TT * P, 255.0, np.float32)
    srel_arr[pos] = srel_s
    srel_host = np.ascontiguousarray(srel_arr.reshape(TT, P).T.astype(BF16))

    xg = np.zeros((TT * P, F), BF16)
    xg[pos] = inp16[tgt_s]
    xgT_host = np.ascontiguousarray(xg.T)  # [P, TT*P] bf16

    sl = np.zeros((NWIN * P, F), np.float32)
    sl[:NPC] = inp[c * NPC : (c + 1) * NPC]
    xs_host = np.ascontiguousarray(
        sl.reshape(NWIN, P, F).transpose(1, 0, 2).reshape(P, NWIN * F)
    )
    return xgT_host, srel_host, xs_host


def _build(T_w, toff, TT, win_of):
    import concourse.bacc as bacc
    import concourse.tile as tile
    from concourse import mybir

    nc = bacc.Bacc(
        "TRN2",
        target_bir_lowering=False,
        debug=False,
        enable_asserts=False,
        num_devices=N_CORES,
    )
    dt = mybir.dt

    max_tw = max(T_w)

    xgT_d = nc.dram_tensor("xgT", [P, TT * P], dt.bfloat16, kind="ExternalInput")
    wT_d = nc.dram_tensor("wT", [P, TF], dt.bfloat16, kind="ExternalInput")
    iota_d = nc.dram_tensor("iota", [P, P], dt.bfloat16, kind="ExternalInput")
    srel_d = nc.dram_tensor("srel", [P, TT], dt.bfloat16, kind="ExternalInput")
    xs_d = nc.dram_tensor("xs", [P, NWIN * F], dt.float32, kind="ExternalInput")
    y_d = nc.dram_tensor("y", [NPC, F], dt.float32, kind="ExternalOutput")

    with tile.TileContext(nc) as tc:
        import contextlib

        with contextlib.ExitStack() as ctx:
            consts = ctx.enter_context(tc.tile_pool(name="consts", bufs=1))
            xg_pool = ctx.enter_context(tc.tile_pool(name="xg", bufs=3))
            ps_pool = ctx.enter_context(tc.tile_pool(name="psGE", bufs=3, space="PSUM"))
            sg_pool = ctx.enter_context(tc.tile_pool(name="sg", bufs=3))
            msg_pool = ctx.enter_context(tc.tile_pool(name="msg", bufs=4))
            oh_pool = ctx.enter_context(tc.tile_pool(name="oh", bufs=2))
            b_ps = ctx.enter_context(tc.tile_pool(name="psB", bufs=2, space="PSUM"))
            out_pool = ctx.enter_context(tc.tile_pool(name="out", bufs=2))

            wT_sb = consts.tile([P, TF], dt.bfloat16, tag="wT")
            nc.sync.dma_start(wT_sb[:], wT_d[:, :])
            iota_sb = consts.tile([P, P], dt.bfloat16, tag="iota")
            nc.sync.dma_start(iota_sb[:], iota_d[:, :])
            srel_sb = consts.tile([P, TT], dt.bfloat16, tag="srel")
            nc.sync.dma_start(srel_sb[:], srel_d[:, :])
            xs_sb = consts.tile([P, NWIN * F], dt.float32, tag="xs")
            nc.sync.dma_start(xs_sb[:], xs_d[:, :])

            state = {"chunk": None, "c0": 0, "oh": None, "psB": None}
            pending = deque()  # (tile_t, msg_tile, col)

            def emit_mm2(t, msg, col):
                w = int(win_of[t])
                i = t - int(toff[w])
                if i == 0:
                    tw = T_w[w]
                    o0 = int(toff[w])
                    oh = oh_pool.tile([P, max_tw * P], dt.bfloat16, tag="oh")
                    nc.vector.tensor_tensor(
                        out=oh[:, : tw * P].rearrange("p (t e) -> p t e", e=P),
                        in0=srel_sb[:, o0 : o0 + tw]
                        .unsqueeze(2)
                        .to_broadcast([P, tw, P]),
                        in1=iota_sb[:].unsqueeze(1).to_broadcast([P, tw, P]),
                        op=mybir.AluOpType.is_equal,
                    )
                    state["oh"] = oh
                    state["psB"] = b_ps.tile(
                        [P, F], dt.float32, tag="psB", name="psB"
                    )
                oh = state["oh"]
                psB = state["psB"]
                nc.tensor.matmul(
                    psB[:],
                    lhsT=oh[:, i * P : (i + 1) * P],
                    rhs=msg[:, col * F : (col + 1) * F],
                    start=(i == 0),
                    stop=(i == T_w[w] - 1),
                )
                if i == T_w[w] - 1:
                    rows = min(P, NPC - w * P)
                    ot = out_pool.tile([P, F], dt.float32, tag="ot")
                    nc.vector.tensor_add(
                        ot[:], psB[:], xs_sb[:, w * F : (w + 1) * F]
                    )
                    nc.sync.dma_start(y_d[w * P : w * P + rows, :], ot[:rows, :])

            for t0 in range(0, TT, PAIR):
                nt = min(PAIR, TT - t0)
                psGE = ps_pool.tile([P, PAIR * TF], dt.float32, tag="psGE")
                for j in range(nt):
                    t = t0 + j
                    if t % CH == 0:
                        cc = min(CH, TT - t)
                        chunk = xg_pool.tile([P, CH * P], dt.bfloat16, tag="xg")
                        nc.sync.dma_start(
                            chunk[:, : cc * P], xgT_d[:, t * P : (t + cc) * P]
                        )
                        state["chunk"] = chunk
                        state["c0"] = t
                    k = t - state["c0"]
                    nc.tensor.matmul(
                        psGE[:, j * TF : (j + 1) * TF],
                        lhsT=state["chunk"][:, k * P : (k + 1) * P],
                        rhs=wT_sb[:],
                        start=True,
                        stop=True,
                    )
                ge = psGE[:].rearrange("p (t c) -> p t c", c=TF)
                sg = sg_pool.tile([P, PAIR * F], dt.bfloat16, tag="sg")
                nc.scalar.activation(
                    sg[:, : nt * F].rearrange("p (t c) -> p t c", c=F),
                    ge[:, :nt, 0:F],
                    mybir.ActivationFunctionType.Sigmoid,
                )
                msg = msg_pool.tile([P, PAIR * F], dt.bfloat16, tag="msg")
                nc.vector.tensor_mul(
                    msg[:, : nt * F].rearrange("p (t c) -> p t c", c=F),
                    ge[:, :nt, F:TF],
                    sg[:, : nt * F].rearrange("p (t c) -> p t c", c=F),
                )
                for j in range(nt):
                    pending.append((t0 + j, msg, j))
                while len(pending) > MM2_LAG:
                    emit_mm2(*pending.popleft())
            while pending:
                emit_mm2(*pending.popleft())

    nc.compile()
    return nc


def _in_maps(plan_arrays, W):
    wT = np.ascontiguousarray(W.T.astype(BF16))
    iota = np.ascontiguousarray(
        np.tile(np.arange(P, dtype=np.float32), (P, 1)).astype(BF16)
    )
    maps = []
    for xgT, srel, xs in plan_arrays:
        maps.append(
            {"xgT": xgT, "wT": wT, "iota": iota, "srel": srel, "xs": xs}
        )
    return maps


def _install_ntff_hook():
    """Provide the antenv.axon_hooks shim trn_boot expects, so trace=True
    can capture NTFF profiles. Silently degrades if anything is missing."""
    try:
        import antenv.axon_hooks  # noqa: F401

        return
    except ImportError:
        pass
    try:
        import types

        import antenv

        mod = types.ModuleType("antenv.axon_hooks")
        _hook = [None]
        mod.set_axon_ntff_profile_hook = lambda h: _hook.__setitem__(0, h)
        mod.get_axon_ntff_profile_hook = lambda: _hook[0]
        sys.modules["antenv.axon_hooks"] = mod
        antenv.axon_hooks = mod
        from trn_agent_boot import trn_boot

        mod.set_axon_ntff_profile_hook(
            trn_boot._ntff_profile_via_ctypes("/opt/axon/libaxon_pjrt.so")
        )
    except Exception:
        pass


def kernel(**inputs) -> np.ndarray:
    inp = np.asarray(inputs["input"], np.float32)
    W = np.asarray(inputs["W"], np.float32)
    es = np.asarray(inputs["edge_sources"]).astype(np.int64)
    et = np.asarray(inputs["edge_targets"]).astype(np.int64)

    T_w, toff, TT, win_of = _plan(es)
    inp16 = inp.astype(BF16)
    plan_arrays = [
        _host_core_arrays(inp16, inp, es, et, toff, TT, c) for c in range(N_CORES)
    ]
    nc = _build(T_w, toff, TT, win_of)

    from concourse.bass_utils import run_bass_kernel_spmd

    if bool(int(os.environ.get("GGC_TRACE", "0"))):
        _install_ntff_hook()
    res = run_bass_kernel_spmd(
        nc,
        _in_maps(plan_arrays, W),
        core_ids=list(range(N_CORES)),
        trace=bool(int(os.environ.get("GGC_TRACE", "0"))),
    )
    out = np.concatenate([res.results[c]["y"] for c in range(N_CORES)], axis=0)
    if bool(int(os.environ.get("GGC_TRACE", "0"))):
        kernel.last_results = res  # stash for test harness
    return out


# revision 8
# speedup vs baseline: 4.0211x; 1.3728x over previous
"""GatedGraphConvolution Trainium2 kernel (host-gather edition).

out = input + segment_sum(sigmoid(g) * e, edge_sources)
  where [g|e] = input[edge_targets] @ W.T

Sharding: edges are sharded by SOURCE node across the 8 cores (6250 nodes
each) so per-core outputs are disjoint and no collectives are needed.  The
host pre-gathers input[edge_targets] for each core's edges ("gathered rows"
per the sharding hint), sorted by 128-node source window and padded to
128-edge tiles.  The host also pre-builds the per-tile one-hot scatter
matrices (exact 0/1 values in bf16) so no engine has to materialize them.

Device per 4-tile group (512 edges):
  mm1 x4: ps[e, 0:256] = xgT_tile.T @ W.T          (PSUM group spans 2 banks)
  act:    sg = sigmoid(ps[:, :, 0:128])             (one instr / 4 tiles)
  mul:    msg = sg * ps[:, :, 128:256]              (DVE, bf16 out)
  mm2 x4: psB[srel, f] += onehot_tile.T @ msg_tile  (PSUM accum per window)
window end: y = psB + x_slice -> DRAM.

No dma_gather (SWDGE descriptor generation measured ~8ns/row on GPSIMD =
854us serial in the M-table design), no M-table round-trip, no collectives.
"""

import math
import os
import sys
from collections import deque

import numpy as np

if "/opt/trn_rl_repo" not in sys.path:
    sys.path.insert(0, "/opt/trn_rl_repo")

import ml_dtypes

P = 128  # partitions / tile edge
F = 128  # feature dim (OUT_F == IN_F == 128)
TF = 2 * F

BF16 = ml_dtypes.bfloat16

N_NODES = 50000
N_CORES = 8
NPC = N_NODES // N_CORES  # 6250
NWIN = math.ceil(NPC / P)  # 49
CH = 16  # tiles per xgT/oh DMA chunk
PAIR = 4  # tiles per PSUM group (psGE [128, 1024] spans 2 banks)
MM2_LAG = 6  # tiles of lag before emitting scatter matmuls


def _plan(edge_sources: np.ndarray):
    """Per-core, per-window tile schedule. Returns (T_w, toff, TT, win_of)."""
    src = edge_sources.astype(np.int64)
    core = src // NPC
    w_all = (src % NPC) // P
    counts = np.zeros((N_CORES, NWIN), np.int64)
    np.add.at(counts, (core, w_all), 1)
    tmax = counts.max(axis=0)
    T_w = [int(math.ceil(tmax[w] / P)) for w in range(NWIN)]
    toff = np.concatenate([[0], np.cumsum(T_w)]).astype(np.int64)
    TT = int(toff[-1])
    win_of = np.zeros(TT, np.int64)
    for w in range(NWIN):
        win_of[toff[w] : toff[w + 1]] = w
    return T_w, toff, TT, win_of


def _host_core_arrays(inp16, edge_sources, edge_targets, toff, TT, c):
    """xgT [P, TT*P] bf16, oh [P, TT*P] bf16, xs [P, NWIN*F] bf16 for core c."""
    src = edge_sources.astype(np.int64)
    tgt = edge_targets.astype(np.int64)
    sel = (src // NPC) == c
    src_c = src[sel] - c * NPC
    tgt_c = tgt[sel]
    w_c = src_c // P
    srel_c = src_c % P

    order = np.argsort(w_c, kind="stable")
    w_s = w_c[order]
    srel_s = srel_c[order]
    tgt_s = tgt_c[order]

    counts = np.bincount(w_s, minlength=NWIN)
    cum = np.concatenate([[0], np.cumsum(counts)])
    within = np.arange(len(w_s)) - cum[w_s]
    pos = toff[w_s] * P + within  # slot index in [0, TT*P)

    xg = np.zeros((TT * P, F), BF16)
    xg[pos] = inp16[tgt_s]
    xgT_host = np.ascontiguousarray(xg.T)  # [P, TT*P] bf16

    ohz = np.zeros((TT * P, P), BF16)
    ohz[pos, srel_s] = 1.0
    oh_host = np.ascontiguousarray(
        ohz.reshape(TT, P, P).transpose(1, 0, 2).reshape(P, TT * P)
    )

    sl = np.zeros((NWIN * P, F), BF16)
    sl[:NPC] = inp16[c * NPC : (c + 1) * NPC]
    xs_host = np.ascontiguousarray(
        sl.reshape(NWIN, P, F).transpose(1, 0, 2).reshape(P, NWIN * F)
    )
    return xgT_host, oh_host, xs_host


def _build(T_w, toff, TT, win_of):
    import concourse.bacc as bacc
    import concourse.tile as tile
    from concourse import mybir

    nc = bacc.Bacc(
        "TRN2",
        target_bir_lowering=False,
        debug=False,
        enable_asserts=False,
        num_devices=N_CORES,
    )
    dt = mybir.dt

    xgT_d = nc.dram_tensor("xgT", [P, TT * P], dt.bfloat16, kind="ExternalInput")
    oh_d = nc.dram_tensor("oh", [P, TT * P], dt.bfloat16, kind="ExternalInput")
    wT_d = nc.dram_tensor("wT", [P, TF], dt.bfloat16, kind="ExternalInput")
    xs_d = nc.dram_tensor("xs", [P, NWIN * F], dt.bfloat16, kind="ExternalInput")
    y_d = nc.dram_tensor("y", [NPC, F], dt.float32, kind="ExternalOutput")

    with tile.TileContext(nc) as tc:
        import contextlib

        with contextlib.ExitStack() as ctx:
            consts = ctx.enter_context(tc.tile_pool(name="consts", bufs=1))
            xg_pool = ctx.enter_context(tc.tile_pool(name="xg", bufs=3))
            ohc_pool = ctx.enter_context(tc.tile_pool(name="ohc", bufs=3))
            ps_pool = ctx.enter_context(tc.tile_pool(name="psGE", bufs=2, space="PSUM"))
            sg_pool = ctx.enter_context(tc.tile_pool(name="sg", bufs=3))
            msg_pool = ctx.enter_context(tc.tile_pool(name="msg", bufs=4))
            b_ps = ctx.enter_context(tc.tile_pool(name="psB", bufs=2, space="PSUM"))
            out_pool = ctx.enter_context(tc.tile_pool(name="out", bufs=2))

            wT_sb = consts.tile([P, TF], dt.bfloat16, tag="wT")
            nc.sync.dma_start(wT_sb[:], wT_d[:, :])
            xs_sb = consts.tile([P, NWIN * F], dt.bfloat16, tag="xs")
            nc.sync.dma_start(xs_sb[:], xs_d[:, :])

            state = {"chunk": None, "ohchunk": None, "c0": 0, "psB": None}
            oh_chunks = {}  # chunk idx -> (tile, start_t)
            pending = deque()  # (tile_t, msg_tile, col)

            def emit_mm2(t, msg, col):
                w = int(win_of[t])
                i = t - int(toff[w])
                if i == 0:
                    state["psB"] = b_ps.tile(
                        [P, F], dt.float32, tag="psB", name="psB"
                    )
                psB = state["psB"]
                ohc, oc0 = oh_chunks[t // CH]
                k = t - oc0
                nc.tensor.matmul(
                    psB[:],
                    lhsT=ohc[:, k * P : (k + 1) * P],
                    rhs=msg[:, col * F : (col + 1) * F],
                    start=(i == 0),
                    stop=(i == T_w[w] - 1),
                )
                if i == T_w[w] - 1:
                    rows = min(P, NPC - w * P)
                    ot = out_pool.tile([P, F], dt.float32, tag="ot")
                    nc.vector.tensor_add(
                        ot[:], psB[:], xs_sb[:, w * F : (w + 1) * F]
                    )
                    nc.sync.dma_start(y_d[w * P : w * P + rows, :], ot[:rows, :])

            for t0 in range(0, TT, PAIR):
                nt = min(PAIR, TT - t0)
                psGE = ps_pool.tile([P, PAIR * TF], dt.float32, tag="psGE")
                for j in range(nt):
                    t = t0 + j
                    if t % CH == 0:
                        cc = min(CH, TT - t)
                        chunk = xg_pool.tile([P, CH * P], dt.bfloat16, tag="xg")
                        nc.sync.dma_start(
                            chunk[:, : cc * P], xgT_d[:, t * P : (t + cc) * P]
                        )
                        ohchunk = ohc_pool.tile(
                            [P, CH * P], dt.bfloat16, tag="ohc"
                        )
                        nc.sync.dma_start(
                            ohchunk[:, : cc * P], oh_d[:, t * P : (t + cc) * P]
                        )
                        state["chunk"] = chunk
                        state["c0"] = t
                        oh_chunks[t // CH] = (ohchunk, t)
                    k = t - state["c0"]
                    nc.tensor.matmul(
                        psGE[:, j * TF : (j + 1) * TF],
                        lhsT=state["chunk"][:, k * P : (k + 1) * P],
                        rhs=wT_sb[:],
                        start=True,
                        stop=True,
                    )
                ge = psGE[:].rearrange("p (t c) -> p t c", c=TF)
                sg = sg_pool.tile([P, PAIR * F], dt.bfloat16, tag="sg")
                nc.scalar.activation(
                    sg[:, : nt * F].rearrange("p (t c) -> p t c", c=F),
                    ge[:, :nt, 0:F],
                    mybir.ActivationFunctionType.Sigmoid,
                )
                msg = msg_pool.tile([P, PAIR * F], dt.bfloat16, tag="msg")
                nc.vector.tensor_mul(
                    msg[:, : nt * F].rearrange("p (t c) -> p t c", c=F),
                    ge[:, :nt, F:TF],
                    sg[:, : nt * F].rearrange("p (t c) -> p t c", c=F),
                )
                for j in range(nt):
                    pending.append((t0 + j, msg, j))
                while len(pending) > MM2_LAG:
                    emit_mm2(*pending.popleft())
            while pending:
                emit_mm2(*pending.popleft())

    nc.compile()
    return nc


def _in_maps(plan_arrays, W):
    wT = np.ascontiguousarray(W.T.astype(BF16))
    maps = []
    for xgT, oh, xs in plan_arrays:
        maps.append({"xgT": xgT, "oh": oh, "wT": wT, "xs": xs})
    return maps


def _install_ntff_hook():
    """Provide the antenv.axon_hooks shim trn_boot expects, so trace=True
    can capture NTFF profiles. Silently degrades if anything is missing."""
    try:
        import antenv.axon_hooks  # noqa: F401

        return
    except ImportError:
        pass
    try:
        import types

        import antenv

        mod = types.ModuleType("antenv.axon_hooks")
        _hook = [None]
        mod.set_axon_ntff_profile_hook = lambda h: _hook.__setitem__(0, h)
        mod.get_axon_ntff_profile_hook = lambda: _hook[0]
        sys.modules["antenv.axon_hooks"] = mod
        antenv.axon_hooks = mod
        from trn_agent_boot import trn_boot

        mod.set_axon_ntff_profile_hook(
            trn_boot._ntff_profile_via_ctypes("/opt/axon/libaxon_pjrt.so")
        )
    except Exception:
        pass


def kernel(**inputs) -> np.ndarray:
    inp = np.asarray(inputs["input"], np.float32)
    W = np.asarray(inputs["W"], np.float32)
    es = np.asarray(inputs["edge_sources"]).astype(np.int64)
    et = np.asarray(inputs["edge_targets"]).astype(np.int64)

    T_w, toff, TT, win_of = _plan(es)
    inp16 = inp.astype(BF16)
    plan_arrays = [
        _host_core_arrays(inp16, es, et, toff, TT, c) for c in range(N_CORES)
    ]
    nc = _build(T_w, toff, TT, win_of)

    from concourse.bass_utils import run_bass_kernel_spmd

    if bool(int(os.environ.get("GGC_TRACE", "0"))):
        _install_ntff_hook()
    res = run_bass_kernel_spmd(
        nc,
        _in_maps(plan_arrays, W),
        core_ids=list(range(N_CORES)),
        trace=bool(int(os.environ.get("GGC_TRACE", "0"))),
    )
    out = np.concatenate([res.results[c]["y"] for c in range(N_CORES)], axis=0)
    if bool(int(os.environ.get("GGC_TRACE", "0"))):
        kernel.last_results = res  # stash for test harness
    return out
